# revision 7
# baseline (speedup 1.0000x reference)
"""Trainium2 Bass kernel for the embedding -> Linear -> tanh-RNN -> Linear -> sigmoid model.

Full-input contract: kernel(**inputs) takes the complete arrays and returns the
complete [128, 1] float32 output. Internally: data-parallel over batch across
8 NeuronCores (16 batch rows per core), weights replicated.

Hardcoded problem shapes:
  x   [128, 512] int   (token ids < 32000)
  emb [32000, 512] f32
  W_w [1024, 512], W_b [1024]
  U_w [1024, 1024], U_b [1024]
  V_w [1, 1024],  V_b [1]
"""

import os
import sys

import numpy as np

sys.path.insert(0, "/opt/trn_rl_repo")

import ml_dtypes  # noqa: E402

import concourse.bass as bass  # noqa: E402
from concourse import bacc  # noqa: E402
import concourse.mybir as mybir  # noqa: E402
import concourse.tile as tile  # noqa: E402
from concourse.bass_utils import run_bass_kernel_spmd  # noqa: E402

B, S, E, H, VOCAB = 128, 512, 512, 1024, 32000
NCORES = 8
BL = B // NCORES  # 16 batch rows per core
NTOK = BL * S  # 8192 tokens per core, flat order i = s*BL + b
CHUNK = 512  # tokens per gather chunk (1024 trips a per-gather limit)
P = 128
ET, HT, KT = E // P, H // P, H // P  # 4, 8, 8

F32 = mybir.dt.float32
BF16 = mybir.dt.bfloat16
I16 = mybir.dt.int16
AF = mybir.ActivationFunctionType

# number of recurrence steps done with matmuls (step 0 is tanh(pre) only)
STEPS = int(os.environ.get("BASS_RNN_STEPS", S))

_cache = {}


def _build():
    nc = bacc.Bacc(None)
    emb_d = nc.declare_dram_parameter("embt", [VOCAB, E], BF16, isOutput=False)
    idx_d = nc.declare_dram_parameter("idx", [P, S], I16, isOutput=False)
    wt_d = nc.declare_dram_parameter("wt", [P, ET, H], BF16, isOutput=False)
    ut_d = nc.declare_dram_parameter("ut", [P, KT, H], BF16, isOutput=False)
    bias_d = nc.declare_dram_parameter("bias", [P, HT], F32, isOutput=False)
    vt_d = nc.declare_dram_parameter("vt", [P, HT], BF16, isOutput=False)
    vb_d = nc.declare_dram_parameter("vb", [1, 1], F32, isOutput=False)
    out_d = nc.declare_dram_parameter("out", [1, BL], F32, isOutput=True)

    with tile.TileContext(nc) as tc:
        with (
            tc.tile_pool(name="const", bufs=1) as constp,
            tc.tile_pool(name="pre", bufs=1) as prep,
            tc.tile_pool(name="xe", bufs=3) as xep,
            tc.tile_pool(name="h", bufs=3) as hp,
            tc.tile_pool(name="misc", bufs=1) as miscp,
        ):
            idx_sb = constp.tile([P, S], I16, tag="idx")
            nc.sync.dma_start(out=idx_sb[:], in_=idx_d[:])
            wt_sb = constp.tile([P, ET, H], BF16, tag="wt")
            nc.sync.dma_start(out=wt_sb[:], in_=wt_d[:])
            ut_sb = constp.tile([P, KT, H], BF16, tag="ut")
            nc.sync.dma_start(out=ut_sb[:], in_=ut_d[:])
            bias_sb = constp.tile([P, HT], F32, tag="bias")
            nc.sync.dma_start(out=bias_sb[:], in_=bias_d[:])
            vt_sb = constp.tile([P, HT], BF16, tag="vt")
            nc.sync.dma_start(out=vt_sb[:], in_=vt_d[:])
            vb_sb = constp.tile([1, 1], F32, tag="vb")
            nc.sync.dma_start(out=vb_sb[:], in_=vb_d[:])

            # preT[p, ht, s*BL + b] = (xe @ W.T + W_b + U_b)[b, s, ht*128 + p]
            preT = prep.tile([P, HT, NTOK], BF16, tag="preT")

            # ---------------- pre-phase ----------------
            with tc.tile_pool(name="pps", bufs=6, space=bass.MemorySpace.PSUM) as psp:
                for c in range(NTOK // CHUNK):
                    xet = xep.tile([P, ET, CHUNK], BF16, tag="xet")
                    nc.gpsimd.dma_gather(
                        out_ap=xet[:],
                        in_ap=emb_d[:],
                        idxs_ap=idx_sb[:, c * (CHUNK // BL) : (c + 1) * (CHUNK // BL)],
                        num_idxs=CHUNK,
                        num_idxs_reg=CHUNK,
                        elem_size=E,
                        transpose=True,
                    )
                    for blk in range(CHUNK // 512):
                        toff = c * CHUNK + blk * 512
                        for ht in range(HT):
                            ps = psp.tile([P, 512], F32, tag="pps")
                            for et in range(ET):
                                nc.tensor.matmul(
                                    ps[:],
                                    wt_sb[:, et, ht * P : (ht + 1) * P],
                                    xet[:, et, blk * 512 : (blk + 1) * 512],
                                    start=(et == 0),
                                    stop=(et == ET - 1),
                                )
                            nc.vector.tensor_tensor(
                                out=preT[:, ht, toff : toff + 512],
                                in0=ps[:],
                                in1=bias_sb[:, ht : ht + 1].to_broadcast([P, 512]),
                                op=mybir.AluOpType.add,
                            )

            # ---------------- recurrence ----------------
            with tc.tile_pool(name="psr", bufs=8, space=bass.MemorySpace.PSUM) as psr:
                h_prev = hp.tile([P, KT, BL], BF16, tag="h")
                for jt in range(HT):
                    nc.scalar.activation(
                        h_prev[:, jt, :], preT[:, jt, 0:BL], AF.Tanh
                    )
                for t in range(1, STEPS):
                    h_new = hp.tile([P, KT, BL], BF16, tag="h")
                    for jt in range(HT):
                        ps = psr.tile([P, BL], F32, tag="psr")
                        for kt in range(KT):
                            nc.tensor.matmul(
                                ps[:],
                                ut_sb[:, kt, jt * P : (jt + 1) * P],
                                h_prev[:, kt, :],
                                start=(kt == 0),
                                stop=(kt == KT - 1),
                            )
                        nc.vector.tensor_tensor(
                            out=ps[:],
                            in0=ps[:],
                            in1=preT[:, jt, t * BL : (t + 1) * BL],
                            op=mybir.AluOpType.add,
                        )
                        nc.scalar.activation(h_new[:, jt, :], ps[:], AF.Tanh)
                    h_prev = h_new

                # ---------------- output head ----------------
                pv = psr.tile([1, BL], F32, tag="psr")
                for kt in range(KT):
                    nc.tensor.matmul(
                        pv[:],
                        vt_sb[:, kt : kt + 1],
                        h_prev[:, kt, :],
                        start=(kt == 0),
                        stop=(kt == KT - 1),
                    )
                out_sb = miscp.tile([1, BL], F32, tag="out")
                nc.scalar.activation(out_sb[:], pv[:], AF.Sigmoid, bias=vb_sb[:])
                nc.sync.dma_start(out=out_d[:], in_=out_sb[:])

    nc.finalize()
    return nc


def kernel(x, emb, W_w, W_b, U_w, U_b, V_w, V_b):
    x = np.asarray(x)
    emb = np.asarray(emb, dtype=np.float32)
    W_w = np.asarray(W_w, dtype=np.float32)
    W_b = np.asarray(W_b, dtype=np.float32)
    U_w = np.asarray(U_w, dtype=np.float32)
    U_b = np.asarray(U_b, dtype=np.float32)
    V_w = np.asarray(V_w, dtype=np.float32)
    V_b = np.asarray(V_b, dtype=np.float32)

    if "nc" not in _cache:
        _cache["nc"] = _build()
    nc = _cache["nc"]

    bf = ml_dtypes.bfloat16
    embt = np.ascontiguousarray(emb.astype(bf))
    # wt[p, et, h] = W_w.T[et*128+p, h]
    wt = np.ascontiguousarray(W_w.T.reshape(ET, P, H).transpose(1, 0, 2).astype(bf))
    # ut[p, kt, j] = U_w.T[kt*128+p, j]
    ut = np.ascontiguousarray(U_w.T.reshape(KT, P, H).transpose(1, 0, 2).astype(bf))
    bias = np.ascontiguousarray((W_b + U_b).reshape(HT, P).T.astype(np.float32))
    vt = np.ascontiguousarray(V_w[0].reshape(HT, P).T.astype(bf))
    vb = V_b.reshape(1, 1).astype(np.float32)

    in_maps = []
    for c in range(NCORES):
        xl = np.ascontiguousarray(
            np.tile(x[c * BL : (c + 1) * BL, :].astype(np.int16), (P // BL, 1))
        )
        in_maps.append(
            {
                "embt": embt,
                "idx": xl,
                "wt": wt,
                "ut": ut,
                "bias": bias,
                "vt": vt,
                "vb": vb,
            }
        )

    _cache["last_in_maps"] = in_maps
    trace = bool(int(os.environ.get("BASS_RNN_TRACE", "0")))
    res = run_bass_kernel_spmd(nc, in_maps, list(range(NCORES)), trace=trace)
    _cache["last_exec_time_ns"] = res.exec_time_ns
    _cache["last_results"] = res

    out = np.empty((B, 1), dtype=np.float32)
    for c in range(NCORES):
        out[c * BL : (c + 1) * BL, 0] = res.results[c]["out"][0, :]
    return out
